# revision 14
# baseline (speedup 1.0000x reference)
"""Polyphase 2x upsample (scatter into one of 4 phases per batch) + circular
3x3 binomial blur, distributed over 8 TRN2 NeuronCores (data-parallel over
batch: 2 batches per core).

Math: with phase p per batch, r = p % 2, c = p // 2, the reference scatters
x[i,j] to y1[2i+r, 2j+c] (zeros elsewhere) and then blurs with
outer([1,2,1],[1,2,1])/16 under circular padding. The output decomposes into
4 parity classes (all indices mod 128, mod 64 inside a pair):
  out[2i+r,   2j+c]   = x[i,j] / 4                    (A sites)
  out[2i+r,   2k+1+c] = (x[i,k] + x[i,k+1]) / 8       (H sites)
  out[2i+1+r, 2j+c]   = (x[i,j] + x[i+1,j]) / 8       (V sites)
  out[2i+1+r, 2k+1+c] = sum of the 4 neighbours / 16  (D sites)
With S  = (x + roll_cols(x))/16 and Sv = (x + roll_rows(x))/16:
  H = 2*S[k] at col (2k+1+c)%128, V = 2*Sv[i] at row (2i+1+r)%128,
  D = Sv[k] + Sv[k+1] at (odd row, (2k+1+c)%128).
All multiplies are powers of two (exact in fp32). Memory-bound shifted-add.

SPMD phase handling (one NEFF for all 8 cores):
 - The column phase bit c selects between two fully static write layouts
   via a runtime 2-arm If per (batch, channel-half) — no per-instruction
   dynamic access patterns (each dynamic-AP instruction permanently burns
   ~2 registers on its engine, so they must stay rare).
 - The row shift by r is folded into the output DMA's DRAM row offsets
   (r, 64+r, (127+r)%128): fixed shapes, dynamic starts, loaded from a tiny
   per-core int32 input via values_load. Dynamic-offset stores are spread
   across the three DMA-capable engines (SP / Act / Pool) to stay under the
   per-engine dynamic-instruction budget.
 - skip_runtime_bounds_check everywhere: the emitted software assert
   instruction faults this runtime.
Work is spread over engines: ACT does the scaled copies (t16, A, H), DVE
the shifted adds (S, Sv, D), GpSimd the V scaled copies.
"""

import sys

for _p in ("/opt/trn_rl_repo",):
    if _p not in sys.path:
        sys.path.insert(0, _p)

import numpy as np

B, C, N = 16, 256, 64
M = 2 * N
NCORES = 8
NB = B // NCORES  # batches per core

_NC_CACHE = None


def _build_nc():
    import concourse.bacc as bacc
    import concourse.bass as bass
    import concourse.mybir as mybir
    import concourse.tile as tile

    f32 = mybir.dt.float32
    i32 = mybir.dt.int32
    add = mybir.AluOpType.add
    mult = mybir.AluOpType.mult
    ds = bass.ds
    ET = mybir.EngineType

    # Bacc (not plain Bass): its finalize() runs generate_event_semaphores,
    # which splits multi-wait instructions — this walrus build allows at
    # most one attached semaphore wait per instruction.
    nc = bacc.Bacc("TRN2", target_bir_lowering=False, debug=False, num_devices=NCORES)
    inp = nc.dram_tensor("inp", [NB, C, N, N], f32, kind="ExternalInput")
    offs = nc.dram_tensor("offs", [1, 16], i32, kind="ExternalInput")
    out = nc.dram_tensor("out", [NB, C, M, M], f32, kind="ExternalOutput")

    with tile.TileContext(nc) as tc:
        with (
            tc.tile_pool(name="offp", bufs=1) as offp,
            tc.tile_pool(name="xp", bufs=2) as xp,
            tc.tile_pool(name="tp", bufs=1) as tp,
            tc.tile_pool(name="op", bufs=3) as op,
        ):
            offs_t = offp.tile([1, 16], i32)
            nc.sync.dma_start(offs_t[:, :], offs[:, :])

            # per batch: [cv, rA, rB, rC] at offs[0, 8*b + k]
            ranges = {
                "cv": (0, 1),    # c
                "rA": (0, 1),    # r
                "rB": (64, 65),  # 64 + r
                "rC": (0, 127),  # (127 + r) % 128
            }
            # cv is the If condition: one value shared by every branching
            # engine. The row offsets are dynamic-AP starts: engine-private
            # loads so their per-use register cost stays on one engine.
            engmap = {
                "cv": ((ET.DVE, ET.Activation, ET.Pool),),
                "rA": ((ET.SP,),),
                "rB": ((ET.Activation,),),
                "rC": ((ET.Pool,),),
            }
            val = {}
            for b in range(NB):
                for k, name in enumerate(("cv", "rA", "rB", "rC")):
                    lo, hi = ranges[name]
                    for engs in engmap[name]:
                        val[(b, name, engs[0])] = nc.values_load(
                            offs_t[0:1, 8 * b + k : 8 * b + k + 1],
                            engines=list(engs),
                            min_val=lo,
                            max_val=hi,
                            skip_runtime_bounds_check=True,
                        )

            def writes(o, q, x, S, Sv, c):
                """Phase-c static column layout for chunk q. Chunk rows are
                tile-relative: local even row 2i' holds A/H of input row
                32q+i', local odd row V/D of the row pair."""
                if c == 0:
                    a_cols = slice(0, 128, 2)        # A/V at cols 2j
                    hm_cols = slice(1, 128, 2)       # H at 2k+1: k=0..63
                    hm_k = slice(0, 64)              # (wrap col folded in S)
                    hw_cols = None
                    dm_cols = slice(1, 126, 2)       # D main 2k+1: k=0..62
                    dw_cols = slice(127, 128)        # D wrap col (k=63)
                else:
                    a_cols = slice(1, 128, 2)        # A/V at cols 2j+1
                    hm_cols = slice(2, 128, 2)       # H at 2k+2: k=0..62
                    hm_k = slice(0, 63)
                    hw_cols = slice(0, 1)            # H wrap col (k=63)
                    dm_cols = slice(2, 128, 2)       # D main 2k+2: k=0..62
                    dw_cols = slice(0, 1)            # D wrap col (k=63)
                rs = slice(32 * q, 32 * q + 32)
                # A sites (ACT): even local rows
                nc.scalar.mul(o[:, 0:64:2, a_cols], x[:, rs, :], 0.25)
                # H sites (ACT): even local rows, H = 2*S
                nc.scalar.mul(o[:, 0:64:2, hm_cols], S[:, rs, hm_k], 2.0)
                if hw_cols is not None:
                    nc.scalar.mul(o[:, 0:64:2, hw_cols], S[:, rs, 63:64], 2.0)
                # V sites (ACT): odd local rows, V = 2*Sv
                nc.scalar.mul(o[:, 1:64:2, a_cols], Sv[:, rs, :], 2.0)
                # D sites (DVE): odd local rows, D = Sv[k] + Sv[k+1]
                nc.vector.tensor_tensor(
                    o[:, 1:64:2, dm_cols],
                    Sv[:, rs, 0:63],
                    Sv[:, rs, 1:64],
                    add,
                )
                nc.vector.tensor_tensor(
                    o[:, 1:64:2, dw_cols],
                    Sv[:, rs, 63:64],
                    Sv[:, rs, 0:1],
                    add,
                )

            for b in range(NB):
                cv = val[(b, "cv", ET.DVE)]
                rA = val[(b, "rA", ET.SP)]
                rB = val[(b, "rB", ET.Activation)]
                rC = val[(b, "rC", ET.Pool)]
                for h in range(C // 128):
                    chs = slice(128 * h, 128 * (h + 1))
                    x = xp.tile([128, N, N], f32, tag="x")
                    nc.sync.dma_start(x[:, :, :], inp[b, chs])

                    t16 = tp.tile([128, N, N], f32, tag="t16")
                    nc.gpsimd.tensor_scalar_mul(t16[:, :, :], x[:, :, :], 0.0625)
                    # S[i,k] = (x[i,k] + x[i,k+1 mod 64]) / 16
                    S = tp.tile([128, N, N], f32, tag="S")
                    nc.vector.tensor_tensor(
                        S[:, :, 0:63], t16[:, :, 0:63], t16[:, :, 1:64], add
                    )
                    nc.vector.tensor_tensor(
                        S[:, :, 63:64], t16[:, :, 63:64], t16[:, :, 0:1], add
                    )
                    # Sv[i,k] = (x[i,k] + x[i+1 mod 64,k]) / 16
                    Sv = tp.tile([128, N, N], f32, tag="Sv")
                    nc.vector.tensor_tensor(
                        Sv[:, 0:63, :], t16[:, 0:63, :], t16[:, 1:64, :], add
                    )
                    nc.vector.tensor_tensor(
                        Sv[:, 63:64, :], t16[:, 63:64, :], t16[:, 0:1, :], add
                    )

                    # per-chunk If so each chunk's store launches as soon
                    # as that chunk is written (shorter discrete units)
                    out3 = out[b, chs]  # [128 ch, 128, 128] DRAM view
                    for q in range(2):
                        o = op.tile([128, 64, M], f32, tag="o", name=f"o_{b}_{h}_{q}")
                        with tc.If(cv < 1) as cmp:
                            writes(o, q, x, S, Sv, 0)
                        with cmp.Else():
                            writes(o, q, x, S, Sv, 1)
                        # store with circular row shift by r folded into DRAM
                        # offsets; dynamic-offset DMAs spread across engines
                        if q == 0:
                            nc.sync.dma_start(out3[:, ds(rA, 64), :], o[:, :, :])
                        else:
                            nc.scalar.dma_start(out3[:, ds(rB, 63), :], o[:, 0:63, :])
                            nc.gpsimd.dma_start(out3[:, ds(rC, 1), :], o[:, 63:64, :])
    return nc


def _get_nc():
    global _NC_CACHE
    if _NC_CACHE is None:
        _NC_CACHE = _build_nc()
    return _NC_CACHE


def _offsets_for(idx_pair):
    offs = np.zeros((1, 16), np.int32)
    for j, p in enumerate(idx_pair):
        p = int(p)
        r, c = p % 2, p // 2
        offs[0, 8 * j : 8 * j + 4] = (c, r, 64 + r, (127 + r) % 128)
    return offs


def kernel(inp, polyphase_indices, _trace=False):
    from concourse.bass_utils import run_bass_kernel_spmd

    inp = np.ascontiguousarray(np.asarray(inp), dtype=np.float32)
    idx = np.asarray(polyphase_indices).astype(np.int32).reshape(B)
    assert inp.shape == (B, C, N, N)

    in_maps = []
    for k in range(NCORES):
        in_maps.append(
            {
                "inp": np.ascontiguousarray(inp[NB * k : NB * (k + 1)]),
                "offs": _offsets_for(idx[NB * k : NB * (k + 1)]),
            }
        )

    nc = _get_nc()
    if not nc.is_finalized():
        nc.finalize()
    res = run_bass_kernel_spmd(
        nc, in_maps, core_ids=list(range(NCORES)), trace=_trace
    )
    out = np.concatenate([res.results[k]["out"] for k in range(NCORES)], axis=0)
    if _trace:
        kernel.last_results = res
    return out


# revision 15
# speedup vs baseline: 2.2488x; 2.2488x over previous
"""Polyphase 2x upsample (scatter into one of 4 phases per batch) + circular
3x3 binomial blur, distributed over 8 TRN2 NeuronCores (data-parallel over
batch: 2 batches per core).

Math: with phase p per batch, r = p % 2, c = p // 2, the reference scatters
x[i,j] to y1[2i+r, 2j+c] (zeros elsewhere) and then blurs with
outer([1,2,1],[1,2,1])/16 under circular padding. The output decomposes into
4 parity classes (all indices mod 128, mod 64 inside a pair):
  out[2i+r,   2j+c]   = x[i,j] / 4                    (A sites)
  out[2i+r,   2k+1+c] = (x[i,k] + x[i,k+1]) / 8       (H sites)
  out[2i+1+r, 2j+c]   = (x[i,j] + x[i+1,j]) / 8       (V sites)
  out[2i+1+r, 2k+1+c] = sum of the 4 neighbours / 16  (D sites)
With S  = (x + roll_cols(x))/16 and Sv = (x + roll_rows(x))/16:
  H = 2*S[k] at col (2k+1+c)%128, V = 2*Sv[i] at row (2i+1+r)%128,
  D = Sv[k] + Sv[k+1] at (odd row, (2k+1+c)%128).
All multiplies are powers of two (exact in fp32). Memory-bound shifted-add.

SPMD phase handling (one NEFF for all 8 cores):
 - The column phase bit c selects between two fully static write layouts
   via a runtime 2-arm If per (batch, channel-half) — no per-instruction
   dynamic access patterns (each dynamic-AP instruction permanently burns
   ~2 registers on its engine, so they must stay rare).
 - The row shift by r is folded into the output DMA's DRAM row offsets
   (r, 64+r, (127+r)%128): fixed shapes, dynamic starts, loaded from a tiny
   per-core int32 input via values_load. Dynamic-offset stores are spread
   across the three DMA-capable engines (SP / Act / Pool) to stay under the
   per-engine dynamic-instruction budget.
 - skip_runtime_bounds_check everywhere: the emitted software assert
   instruction faults this runtime.
Work is spread over engines: ACT does the scaled copies (t16, A, H), DVE
the shifted adds (S, Sv, D), GpSimd the V scaled copies.
"""

import sys

for _p in ("/opt/trn_rl_repo",):
    if _p not in sys.path:
        sys.path.insert(0, _p)

import numpy as np

B, C, N = 16, 256, 64
M = 2 * N
NCORES = 8
NB = B // NCORES  # batches per core

_NC_CACHE = None


def _build_nc():
    import concourse.bacc as bacc
    import concourse.bass as bass
    import concourse.mybir as mybir
    import concourse.tile as tile

    f32 = mybir.dt.float32
    i32 = mybir.dt.int32
    add = mybir.AluOpType.add
    mult = mybir.AluOpType.mult
    ds = bass.ds
    ET = mybir.EngineType

    # Bacc (not plain Bass): its finalize() runs generate_event_semaphores,
    # which splits multi-wait instructions — this walrus build allows at
    # most one attached semaphore wait per instruction.
    nc = bacc.Bacc("TRN2", target_bir_lowering=False, debug=False, num_devices=NCORES)
    inp = nc.dram_tensor("inp", [NB, C, N, N], f32, kind="ExternalInput")
    offs = nc.dram_tensor("offs", [1, 16], i32, kind="ExternalInput")
    out = nc.dram_tensor("out", [NB, C, M, M], f32, kind="ExternalOutput")

    with tile.TileContext(nc) as tc:
        with (
            tc.tile_pool(name="offp", bufs=1) as offp,
            tc.tile_pool(name="xp", bufs=2) as xp,
            tc.tile_pool(name="tp", bufs=1) as tp,
            tc.tile_pool(name="op", bufs=3) as op,
        ):
            offs_t = offp.tile([1, 16], i32)
            nc.sync.dma_start(offs_t[:, :], offs[:, :])

            # per batch: [cv, rA, rB, rC] at offs[0, 8*b + k]
            ranges = {
                "cv": (0, 1),    # c
                "rA": (0, 1),    # r
                "rB": (64, 65),  # 64 + r
                "rC": (0, 127),  # (127 + r) % 128
            }
            # cv is the If condition: one value shared by every branching
            # engine. The row offsets are dynamic-AP starts: engine-private
            # loads so their per-use register cost stays on one engine.
            engmap = {
                "cv": ((ET.DVE, ET.Activation, ET.Pool),),
                "rA": ((ET.SP,),),
                "rB": ((ET.Activation,),),
                "rC": ((ET.Pool,),),
            }
            val = {}
            for b in range(NB):
                for k, name in enumerate(("cv", "rA", "rB", "rC")):
                    lo, hi = ranges[name]
                    for engs in engmap[name]:
                        val[(b, name, engs[0])] = nc.values_load(
                            offs_t[0:1, 8 * b + k : 8 * b + k + 1],
                            engines=list(engs),
                            min_val=lo,
                            max_val=hi,
                            skip_runtime_bounds_check=True,
                        )

            def writes(o, q, x, S, Sv, c):
                """Phase-c static column layout for chunk q. Chunk rows are
                tile-relative: local even row 2i' holds A/H of input row
                32q+i', local odd row V/D of the row pair."""
                if c == 0:
                    a_cols = slice(0, 128, 2)        # A/V at cols 2j
                    hm_cols = slice(1, 128, 2)       # H at 2k+1: k=0..63
                    hm_k = slice(0, 64)              # (wrap col folded in S)
                    hw_cols = None
                    dm_cols = slice(1, 126, 2)       # D main 2k+1: k=0..62
                    dw_cols = slice(127, 128)        # D wrap col (k=63)
                else:
                    a_cols = slice(1, 128, 2)        # A/V at cols 2j+1
                    hm_cols = slice(2, 128, 2)       # H at 2k+2: k=0..62
                    hm_k = slice(0, 63)
                    hw_cols = slice(0, 1)            # H wrap col (k=63)
                    dm_cols = slice(2, 128, 2)       # D main 2k+2: k=0..62
                    dw_cols = slice(0, 1)            # D wrap col (k=63)
                rs = slice(32 * q, 32 * q + 32)
                # A sites (ACT): even local rows
                nc.scalar.mul(o[:, 0:64:2, a_cols], x[:, rs, :], 0.25)
                # H sites (ACT): even local rows, H = 2*S
                nc.scalar.mul(o[:, 0:64:2, hm_cols], S[:, rs, hm_k], 2.0)
                if hw_cols is not None:
                    nc.scalar.mul(o[:, 0:64:2, hw_cols], S[:, rs, 63:64], 2.0)
                # V sites (ACT): odd local rows, V = 2*Sv
                nc.scalar.mul(o[:, 1:64:2, a_cols], Sv[:, rs, :], 2.0)
                # D sites (DVE): odd local rows, D = Sv[k] + Sv[k+1]
                nc.vector.tensor_tensor(
                    o[:, 1:64:2, dm_cols],
                    Sv[:, rs, 0:63],
                    Sv[:, rs, 1:64],
                    add,
                )
                nc.vector.tensor_tensor(
                    o[:, 1:64:2, dw_cols],
                    Sv[:, rs, 63:64],
                    Sv[:, rs, 0:1],
                    add,
                )

            for b in range(NB):
                cv = val[(b, "cv", ET.DVE)]
                rA = val[(b, "rA", ET.SP)]
                rB = val[(b, "rB", ET.Activation)]
                rC = val[(b, "rC", ET.Pool)]
                for h in range(C // 128):
                    chs = slice(128 * h, 128 * (h + 1))
                    x = xp.tile([128, N, N], f32, tag="x")
                    nc.sync.dma_start(x[:, :, :], inp[b, chs])

                    t16 = tp.tile([128, N, N], f32, tag="t16")
                    nc.scalar.mul(t16[:, :, :], x[:, :, :], 0.0625)
                    # S[i,k] = (x[i,k] + x[i,k+1 mod 64]) / 16
                    S = tp.tile([128, N, N], f32, tag="S")
                    nc.vector.tensor_tensor(
                        S[:, :, 0:63], t16[:, :, 0:63], t16[:, :, 1:64], add
                    )
                    nc.vector.tensor_tensor(
                        S[:, :, 63:64], t16[:, :, 63:64], t16[:, :, 0:1], add
                    )
                    # Sv[i,k] = (x[i,k] + x[i+1 mod 64,k]) / 16
                    Sv = tp.tile([128, N, N], f32, tag="Sv")
                    nc.vector.tensor_tensor(
                        Sv[:, 0:63, :], t16[:, 0:63, :], t16[:, 1:64, :], add
                    )
                    nc.vector.tensor_tensor(
                        Sv[:, 63:64, :], t16[:, 63:64, :], t16[:, 0:1, :], add
                    )

                    # per-chunk If so each chunk's store launches as soon
                    # as that chunk is written (shorter discrete units)
                    out3 = out[b, chs]  # [128 ch, 128, 128] DRAM view
                    for q in range(2):
                        o = op.tile([128, 64, M], f32, tag="o", name=f"o_{b}_{h}_{q}")
                        with tc.If(cv < 1) as cmp:
                            writes(o, q, x, S, Sv, 0)
                        with cmp.Else():
                            writes(o, q, x, S, Sv, 1)
                        # store with circular row shift by r folded into DRAM
                        # offsets; dynamic-offset DMAs spread across engines
                        if q == 0:
                            nc.sync.dma_start(out3[:, ds(rA, 64), :], o[:, :, :])
                        else:
                            nc.scalar.dma_start(out3[:, ds(rB, 63), :], o[:, 0:63, :])
                            nc.gpsimd.dma_start(out3[:, ds(rC, 1), :], o[:, 63:64, :])
    return nc


def _get_nc():
    global _NC_CACHE
    if _NC_CACHE is None:
        _NC_CACHE = _build_nc()
    return _NC_CACHE


def _offsets_for(idx_pair):
    offs = np.zeros((1, 16), np.int32)
    for j, p in enumerate(idx_pair):
        p = int(p)
        r, c = p % 2, p // 2
        offs[0, 8 * j : 8 * j + 4] = (c, r, 64 + r, (127 + r) % 128)
    return offs


def kernel(inp, polyphase_indices, _trace=False):
    from concourse.bass_utils import run_bass_kernel_spmd

    inp = np.ascontiguousarray(np.asarray(inp), dtype=np.float32)
    idx = np.asarray(polyphase_indices).astype(np.int32).reshape(B)
    assert inp.shape == (B, C, N, N)

    in_maps = []
    for k in range(NCORES):
        in_maps.append(
            {
                "inp": np.ascontiguousarray(inp[NB * k : NB * (k + 1)]),
                "offs": _offsets_for(idx[NB * k : NB * (k + 1)]),
            }
        )

    nc = _get_nc()
    if not nc.is_finalized():
        nc.finalize()
    res = run_bass_kernel_spmd(
        nc, in_maps, core_ids=list(range(NCORES)), trace=_trace
    )
    out = np.concatenate([res.results[k]["out"] for k in range(NCORES)], axis=0)
    if _trace:
        kernel.last_results = res
    return out


# revision 16
# speedup vs baseline: 2.2726x; 1.0106x over previous
"""Polyphase 2x upsample (scatter into one of 4 phases per batch) + circular
3x3 binomial blur, distributed over 8 TRN2 NeuronCores (data-parallel over
batch: 2 batches per core).

Math: with phase p per batch, r = p % 2, c = p // 2, the reference scatters
x[i,j] to y1[2i+r, 2j+c] (zeros elsewhere) and then blurs with
outer([1,2,1],[1,2,1])/16 under circular padding. The output decomposes into
4 parity classes (all indices mod 128, mod 64 inside a pair):
  out[2i+r,   2j+c]   = x[i,j] / 4                    (A sites)
  out[2i+r,   2k+1+c] = (x[i,k] + x[i,k+1]) / 8       (H sites)
  out[2i+1+r, 2j+c]   = (x[i,j] + x[i+1,j]) / 8       (V sites)
  out[2i+1+r, 2k+1+c] = sum of the 4 neighbours / 16  (D sites)
With S  = (x + roll_cols(x))/16 and Sv = (x + roll_rows(x))/16:
  H = 2*S[k] at col (2k+1+c)%128, V = 2*Sv[i] at row (2i+1+r)%128,
  D = Sv[k] + Sv[k+1] at (odd row, (2k+1+c)%128).
All multiplies are powers of two (exact in fp32). Memory-bound shifted-add.

SPMD phase handling (one NEFF for all 8 cores):
 - The column phase bit c selects between two fully static write layouts
   via a runtime 2-arm If per (batch, channel-half) — no per-instruction
   dynamic access patterns (each dynamic-AP instruction permanently burns
   ~2 registers on its engine, so they must stay rare).
 - The row shift by r is folded into the output DMA's DRAM row offsets
   (r, 64+r, (127+r)%128): fixed shapes, dynamic starts, loaded from a tiny
   per-core int32 input via values_load. Dynamic-offset stores are spread
   across the three DMA-capable engines (SP / Act / Pool) to stay under the
   per-engine dynamic-instruction budget.
 - skip_runtime_bounds_check everywhere: the emitted software assert
   instruction faults this runtime.
Work is spread over engines: ACT does the scaled copies (t16, A, H), DVE
the shifted adds (S, Sv, D), GpSimd the V scaled copies.
"""

import sys

for _p in ("/opt/trn_rl_repo",):
    if _p not in sys.path:
        sys.path.insert(0, _p)

import numpy as np

B, C, N = 16, 256, 64
M = 2 * N
NCORES = 8
NB = B // NCORES  # batches per core

_NC_CACHE = None


def _build_nc():
    import concourse.bacc as bacc
    import concourse.bass as bass
    import concourse.mybir as mybir
    import concourse.tile as tile

    f32 = mybir.dt.float32
    i32 = mybir.dt.int32
    add = mybir.AluOpType.add
    mult = mybir.AluOpType.mult
    ds = bass.ds
    ET = mybir.EngineType

    # Bacc (not plain Bass): its finalize() runs generate_event_semaphores,
    # which splits multi-wait instructions — this walrus build allows at
    # most one attached semaphore wait per instruction.
    nc = bacc.Bacc("TRN2", target_bir_lowering=False, debug=False, num_devices=NCORES)
    inp = nc.dram_tensor("inp", [NB, C, N, N], f32, kind="ExternalInput")
    offs = nc.dram_tensor("offs", [1, 16], i32, kind="ExternalInput")
    out = nc.dram_tensor("out", [NB, C, M, M], f32, kind="ExternalOutput")

    with tile.TileContext(nc) as tc:
        with (
            tc.tile_pool(name="offp", bufs=1) as offp,
            tc.tile_pool(name="xp", bufs=2) as xp,
            tc.tile_pool(name="tp", bufs=1) as tp,
            tc.tile_pool(name="op", bufs=3) as op,
        ):
            offs_t = offp.tile([1, 16], i32)
            nc.sync.dma_start(offs_t[:, :], offs[:, :])

            # per batch: [cv, rA, rB, rC] at offs[0, 8*b + k]
            ranges = {
                "cv": (0, 1),    # c
                "rA": (0, 1),    # r
                "rB": (64, 65),  # 64 + r
                "rC": (0, 127),  # (127 + r) % 128
            }
            # cv is the If condition: one value shared by every branching
            # engine. The row offsets are dynamic-AP starts: engine-private
            # loads so their per-use register cost stays on one engine.
            engmap = {
                "cv": ((ET.DVE, ET.Activation, ET.Pool),),
                "rA": ((ET.SP,),),
                "rB": ((ET.Activation,),),
                "rC": ((ET.Pool,),),
            }
            val = {}
            for b in range(NB):
                for k, name in enumerate(("cv", "rA", "rB", "rC")):
                    lo, hi = ranges[name]
                    for engs in engmap[name]:
                        val[(b, name, engs[0])] = nc.values_load(
                            offs_t[0:1, 8 * b + k : 8 * b + k + 1],
                            engines=list(engs),
                            min_val=lo,
                            max_val=hi,
                            skip_runtime_bounds_check=True,
                        )

            def writes(o, q, x, S, Sv, c):
                """Phase-c static column layout for chunk q. Chunk rows are
                tile-relative: local even row 2i' holds A/H of input row
                32q+i', local odd row V/D of the row pair."""
                if c == 0:
                    a_cols = slice(0, 128, 2)        # A/V at cols 2j
                    hm_cols = slice(1, 128, 2)       # H at 2k+1: k=0..63
                    hm_k = slice(0, 64)              # (wrap col folded in S)
                    hw_cols = None
                    dm_cols = slice(1, 126, 2)       # D main 2k+1: k=0..62
                    dw_cols = slice(127, 128)        # D wrap col (k=63)
                else:
                    a_cols = slice(1, 128, 2)        # A/V at cols 2j+1
                    hm_cols = slice(2, 128, 2)       # H at 2k+2: k=0..62
                    hm_k = slice(0, 63)
                    hw_cols = slice(0, 1)            # H wrap col (k=63)
                    dm_cols = slice(2, 128, 2)       # D main 2k+2: k=0..62
                    dw_cols = slice(0, 1)            # D wrap col (k=63)
                rs = slice(32 * q, 32 * q + 32)
                # A sites (ACT): even local rows
                nc.scalar.mul(o[:, 0:64:2, a_cols], x[:, rs, :], 0.25)
                # H sites (ACT): even local rows, H = 2*S
                nc.scalar.mul(o[:, 0:64:2, hm_cols], S[:, rs, hm_k], 2.0)
                if hw_cols is not None:
                    nc.scalar.mul(o[:, 0:64:2, hw_cols], S[:, rs, 63:64], 2.0)
                # V sites (ACT): odd local rows, V = 2*Sv
                nc.scalar.mul(o[:, 1:64:2, a_cols], Sv[:, rs, :], 2.0)
                # D sites (DVE): odd local rows, D = Sv[k] + Sv[k+1]
                nc.vector.tensor_tensor(
                    o[:, 1:64:2, dm_cols],
                    Sv[:, rs, 0:63],
                    Sv[:, rs, 1:64],
                    add,
                )
                nc.vector.tensor_tensor(
                    o[:, 1:64:2, dw_cols],
                    Sv[:, rs, 63:64],
                    Sv[:, rs, 0:1],
                    add,
                )

            for b in range(NB):
                cv = val[(b, "cv", ET.DVE)]
                rA = val[(b, "rA", ET.SP)]
                rB = val[(b, "rB", ET.Activation)]
                rC = val[(b, "rC", ET.Pool)]
                for h in range(C // 128):
                    chs = slice(128 * h, 128 * (h + 1))
                    x = xp.tile([128, N, N], f32, tag="x")
                    nc.sync.dma_start(x[:, :, :], inp[b, chs])

                    t16 = tp.tile([128, N, N], f32, tag="t16")
                    nc.scalar.mul(t16[:, :, :], x[:, :, :], 0.0625)
                    # S[i,k] = (x[i,k] + x[i,k+1 mod 64]) / 16
                    S = tp.tile([128, N, N], f32, tag="S")
                    nc.vector.tensor_tensor(
                        S[:, :, 0:63], t16[:, :, 0:63], t16[:, :, 1:64], add
                    )
                    nc.vector.tensor_tensor(
                        S[:, :, 63:64], t16[:, :, 63:64], t16[:, :, 0:1], add
                    )
                    # Sv[i,k] = (x[i,k] + x[i+1 mod 64,k]) / 16
                    Sv = tp.tile([128, N, N], f32, tag="Sv")
                    nc.vector.tensor_tensor(
                        Sv[:, 0:63, :], t16[:, 0:63, :], t16[:, 1:64, :], add
                    )
                    nc.vector.tensor_tensor(
                        Sv[:, 63:64, :], t16[:, 63:64, :], t16[:, 0:1, :], add
                    )

                    out3 = out[b, chs]  # [128 ch, 128, 128] DRAM view
                    o2 = [
                        op.tile([128, 64, M], f32, tag="o", name=f"o_{b}_{h}_{q}")
                        for q in range(2)
                    ]
                    with tc.If(cv < 1) as cmp:
                        for q in range(2):
                            writes(o2[q], q, x, S, Sv, 0)
                    with cmp.Else():
                        for q in range(2):
                            writes(o2[q], q, x, S, Sv, 1)
                    # store with circular row shift by r folded into DRAM
                    # offsets; dynamic-offset DMAs spread across engines
                    nc.sync.dma_start(out3[:, ds(rA, 64), :], o2[0][:, :, :])
                    nc.scalar.dma_start(out3[:, ds(rB, 63), :], o2[1][:, 0:63, :])
                    nc.gpsimd.dma_start(out3[:, ds(rC, 1), :], o2[1][:, 63:64, :])
    return nc


def _get_nc():
    global _NC_CACHE
    if _NC_CACHE is None:
        _NC_CACHE = _build_nc()
    return _NC_CACHE


def _offsets_for(idx_pair):
    offs = np.zeros((1, 16), np.int32)
    for j, p in enumerate(idx_pair):
        p = int(p)
        r, c = p % 2, p // 2
        offs[0, 8 * j : 8 * j + 4] = (c, r, 64 + r, (127 + r) % 128)
    return offs


def kernel(inp, polyphase_indices, _trace=False):
    from concourse.bass_utils import run_bass_kernel_spmd

    inp = np.ascontiguousarray(np.asarray(inp), dtype=np.float32)
    idx = np.asarray(polyphase_indices).astype(np.int32).reshape(B)
    assert inp.shape == (B, C, N, N)

    in_maps = []
    for k in range(NCORES):
        in_maps.append(
            {
                "inp": np.ascontiguousarray(inp[NB * k : NB * (k + 1)]),
                "offs": _offsets_for(idx[NB * k : NB * (k + 1)]),
            }
        )

    nc = _get_nc()
    if not nc.is_finalized():
        nc.finalize()
    res = run_bass_kernel_spmd(
        nc, in_maps, core_ids=list(range(NCORES)), trace=_trace
    )
    out = np.concatenate([res.results[k]["out"] for k in range(NCORES)], axis=0)
    if _trace:
        kernel.last_results = res
    return out
